# revision 40
# baseline (speedup 1.0000x reference)
# Trainium2 Bass kernel for nn_CustomStyleLoss (segment-mean + MSE reduction).
#
# loss = sum_rows mean_chunks( (mean_chunk(input) - mean_chunk(style))^2 )
# with rows = 16*512 = 8192, each row = 50*50 = 2500 elems = 25 chunks of 100.
#
# Data-parallel over the row axis: core i gets rows [i*1024, (i+1)*1024).
# Raw Bass (no Tile framework). Per core: 8 row-tiles of 128 rows.
#
# DMA: each tile's 2500 columns stream as THREE pieces (1000/1000/500)
# per tensor — input on the SP HWDGE ring, style on the ACT ring. The
# two rings hold ~400 GB/s combined with 4KB lines (10KB lines only
# reach ~352 GB/s — DMA piece size is a throughput knob). Adding a third
# SWDGE queue on GpSimd was tried and made things WORSE (84.9us vs
# 68.7us): SWDGE packets steal SDMA-engine slots and its descriptor
# rings contend for SBUF AXI ports, slowing both HWDGE rings. All
# issues are emitted before any compute waits on the issuing engine: the
# issue instruction stalls on ring-full backpressure, and a compute wait
# queued behind it would starve the stream.
#
# DVE: per tile just TWO chained tensor_tensor_scans (cols 0:2000 once
# the first two pieces land, then 2000:2500 chaining through col 2000) —
# running sum of (input - style), fp32 state, over a [128, 2501] buffer
# whose col 0 is a permanent zero — followed by ONE strided 25-column sub
# for the chunk sums. ~6.2us/tile keeps the DVE just under the stream
# cadence, and the 500-column final piece keeps the post-stream tail
# near ~2us.
#
# ACT: squares+reduces each tile's 25 chunk sums into partials[:, t]
# (fused activation accumulate), then ships partials via its own ring.
# The DVE never waits on ACT (cs has one slot per tile), so ring
# backpressure on the issuing engines can never reach the critical path.
# The loss scale is applied on the host.

import sys

if "/opt/trn_rl_repo" not in sys.path:
    sys.path.insert(0, "/opt/trn_rl_repo")

import numpy as np

import concourse.bass as bass
from concourse import mybir
from concourse.bass_utils import run_bass_kernel_spmd

N_CORES = 8
N_ROWS = 8192          # 16 * 512
K = 2500               # 50 * 50
CHUNK = 100
P = 128
ROWS_PER_CORE = N_ROWS // N_CORES   # 1024
ROWS_PER_TILE = P                   # 128
N_TILES = ROWS_PER_CORE // ROWS_PER_TILE  # 8
N_BUFS = 7
# DMA pieces per tile (chunk-aligned); scan pieces group them for the DVE.
DMA_PIECES = [(0, 1000), (1000, 2000), (2000, 2500)]
N_PIECES = len(DMA_PIECES)
# (col start, col end, dma piece indices that must have landed).
# Tile 0 scans piece-by-piece so the first scan starts as early as
# possible; later tiles fuse pieces 0+1 into one scan (fewer fixed costs).
SCAN_PIECES = [(0, 2000, (0, 1)), (2000, 2500, (2,))]
SCAN_PIECES_T0 = [(0, 1000, (0,)), (1000, 2000, (1,)), (2000, 2500, (2,))]
SCALE = 1.0 / (CHUNK * np.sqrt(K // CHUNK))
SCALE2 = float(SCALE * SCALE)

_CACHED_NC = None


def _build_nc():
    nc = bass.Bass(
        "TRN2",
        target_bir_lowering=False,
        debug=False,
        num_devices=N_CORES,
    )
    x = nc.dram_tensor(
        "input", [ROWS_PER_CORE, K], mybir.dt.float32, kind="ExternalInput"
    ).ap()
    s = nc.dram_tensor(
        "style", [ROWS_PER_CORE, K], mybir.dt.float32, kind="ExternalInput"
    ).ap()
    o = nc.dram_tensor(
        "out", [P, N_TILES], mybir.dt.float32, kind="ExternalOutput"
    ).ap()

    def src(t_ap, t, c0, c1):
        return t_ap[t * ROWS_PER_TILE : (t + 1) * ROWS_PER_TILE, c0:c1]

    from contextlib import ExitStack

    with ExitStack() as ctx:
        xt = ctx.enter_context(
            nc.sbuf_tensor("xt", [P, N_BUFS, K], mybir.dt.float32)
        )
        st = ctx.enter_context(
            nc.sbuf_tensor("st", [P, N_BUFS, K], mybir.dt.float32)
        )
        # One scan buffer for the whole tile; scan pieces chain via
        # initial=prev[:, -1:], col 0 is a permanent zero, and all 25
        # chunk sums come from ONE strided sub.
        scb = ctx.enter_context(
            nc.sbuf_tensor("scb", [P, K + 1], mybir.dt.float32)
        )
        # One cs slot per tile (tiny) so the DVE NEVER waits on the ACT
        # squares — the issuing engines stall on ring-full backpressure
        # for multiple microseconds, and any DVE->ACT coupling would pull
        # that stall into the critical path.
        cs = ctx.enter_context(
            nc.sbuf_tensor("cs", [P, N_TILES, K // CHUNK], mybir.dt.float32)
        )
        sq = ctx.enter_context(
            nc.sbuf_tensor("sq", [P, K // CHUNK], mybir.dt.float32)
        )
        sqv = ctx.enter_context(
            nc.sbuf_tensor("sqv", [P, K // CHUNK], mybir.dt.float32)
        )
        partials = ctx.enter_context(
            nc.sbuf_tensor("partials", [P, N_TILES], mybir.dt.float32)
        )
        # One semaphore per DMA so no completion-ordering assumptions are
        # needed between DMAs on the same queue.
        s_in = [
            [
                ctx.enter_context(nc.semaphore(f"s_in{t}_{p}"))
                for p in range(N_PIECES)
            ]
            for t in range(N_TILES)
        ]
        s_st = [
            [
                ctx.enter_context(nc.semaphore(f"s_st{t}_{p}"))
                for p in range(N_PIECES)
            ]
            for t in range(N_TILES)
        ]
        s_sub = ctx.enter_context(nc.semaphore("s_sub"))
        s_fin = ctx.enter_context(nc.semaphore("s_fin"))
        s_out = ctx.enter_context(nc.semaphore("s_out"))
        block = ctx.enter_context(nc.Block(no_gpsimd_drain=True))

        @block.sync
        def _(sync):
            # Input pieces 0/1 on the SP HWDGE ring. The first N_BUFS
            # tiles issue immediately; tile t >= N_BUFS reuses slot
            # t % N_BUFS, free once tile t - N_BUFS's scans+sub are done.
            for t in range(N_TILES):
                if t >= N_BUFS:
                    sync.wait_ge(s_sub, t - N_BUFS + 1)
                for p, (c0, c1) in enumerate(DMA_PIECES):
                    sync.dma_start(
                        out=xt[:, t % N_BUFS, c0:c1], in_=src(x, t, c0, c1)
                    ).then_inc(s_in[t][p], 16)

        @block.scalar
        def _(scalar):
            # Style pieces on the ACT HWDGE ring. All issues come first —
            # a compute wait interleaved between issues would stall the
            # ring on its semaphore and starve the stream.
            for t in range(N_TILES):
                if t >= N_BUFS:
                    scalar.wait_ge(s_sub, t - N_BUFS + 1)
                for p, (c0, c1) in enumerate(DMA_PIECES):
                    scalar.dma_start(
                        out=st[:, t % N_BUFS, c0:c1], in_=src(s, t, c0, c1)
                    ).then_inc(s_st[t][p], 16)
            # partials[:, t] = sum_c cs[:, t, c]^2 — fused square+reduce
            # on the ACT engine for tiles 0..6, so the DVE only runs
            # scans and subs mid-stream. Tile 7's square+reduce runs on
            # the DVE instead: on the critical tail, that skips the
            # sub -> ACT handoff and the ACT drain.
            for t in range(N_TILES - 1):
                scalar.wait_ge(s_sub, t + 1)
                nc.scalar.activation(
                    out=sq[:],
                    in_=cs[:, t, :],
                    func=mybir.ActivationFunctionType.Square,
                    accum_out=partials[:, t : t + 1],
                )
            # Make this engine's own accumulate writes visible, then wait
            # for the DVE's tile-7 reduce and ship the partials. No wait
            # on the out-DMA receipt: the engine postamble + NRT teardown
            # give the 4KB write ample time to land in DRAM before the
            # host reads the output.
            scalar.drain()
            scalar.wait_ge(s_fin, 1)
            scalar.dma_start(out=o, in_=partials[:]).then_inc(s_out, 16)

        @block.vector
        def _(vector):
            nc.vector.memset(scb[:, 0:1], 0.0)
            for t in range(N_TILES):
                slot = t % N_BUFS
                for c0, c1, deps in (SCAN_PIECES_T0 if t == 0 else SCAN_PIECES):
                    for p in deps:
                        vector.wait_ge(s_in[t][p], 16)
                        vector.wait_ge(s_st[t][p], 16)
                    # scb[:, j] = sum_{i<=j} (xt[:, i] - st[:, i]); scan
                    # pieces chain through the previous piece's last col.
                    nc.vector.tensor_tensor_scan(
                        out=scb[:, c0 + 1 : c1 + 1],
                        data0=xt[:, slot, c0:c1],
                        data1=st[:, slot, c0:c1],
                        initial=0.0 if c0 == 0 else scb[:, c0 : c0 + 1],
                        op0=mybir.AluOpType.add,
                        op1=mybir.AluOpType.subtract,
                    )
                    vector.drain()
                # chunk sums: cs[c] = S[100(c+1)] - S[100c]  (S[0] == 0).
                # Completion frees the xt/st slot and hands cs[:, t] to
                # the ACT square.
                nc.vector.tensor_sub(
                    cs[:, t, :],
                    scb[:, CHUNK : K + 1 : CHUNK],
                    scb[:, 0:K:CHUNK],
                ).then_inc(s_sub, 1)
            # Tile 7's square+reduce on the DVE (sqv is DVE-private; sq
            # belongs to the ACT squares).
            last = N_TILES - 1
            vector.drain()
            nc.vector.tensor_mul(sqv[:], cs[:, last, :], cs[:, last, :])
            vector.drain()
            nc.vector.tensor_reduce(
                out=partials[:, last : last + 1],
                in_=sqv[:],
                axis=mybir.AxisListType.X,
                op=mybir.AluOpType.add,
            ).then_inc(s_fin, 1)

    return nc


def _get_nc():
    global _CACHED_NC
    if _CACHED_NC is None:
        _CACHED_NC = _build_nc()
    return _CACHED_NC


def run_sharded(input, style, **run_kwargs):
    """Shard, run on 8 cores, return (scalar loss, BassKernelResults)."""
    nc = _get_nc()
    xi = np.ascontiguousarray(np.asarray(input, dtype=np.float32)).reshape(
        N_ROWS, K
    )
    xs = np.ascontiguousarray(np.asarray(style, dtype=np.float32)).reshape(
        N_ROWS, K
    )
    in_maps = [
        {
            "input": xi[i * ROWS_PER_CORE : (i + 1) * ROWS_PER_CORE],
            "style": xs[i * ROWS_PER_CORE : (i + 1) * ROWS_PER_CORE],
        }
        for i in range(N_CORES)
    ]
    res = run_bass_kernel_spmd(nc, in_maps, list(range(N_CORES)), **run_kwargs)
    total = np.float64(0.0)
    for r in res.results:
        total += r["out"].astype(np.float64).sum()
    return np.array(total * SCALE2, dtype=np.float32), res


def kernel(input, style):
    loss, _ = run_sharded(input, style)
    return loss
